# revision 8
# baseline (speedup 1.0000x reference)
"""Trainium2 Bass kernel for nn_BeliefPropagationTorch.

Structure of the computation (faithful to the reference):
  log_msg_0 = 0
  repeat 15x:
    inc      = adjT @ trunc(log_msg)
    log_temp = log_psi + phi_in + inc
    new      = 0.5*logsumexp(log_temp, axis=2) + 0.5*log_msg
    new      = new - logsumexp(new, axis=1)
    prob0_t  = sigmoid(2b + segment_sum(new0-new1, edge_out))
  outputs: exp(log_softmax(log_phi + segment_sum(new))), KL loss, prob_step

Key structural fact used by the fast path: log_msg is a per-edge
log-softmax over 2 states, so every component lies in (-LSE, 0].  While
every component stays in (-1, 0], trunc(log_msg) == 0 elementwise, so
inc == adjT @ 0 == 0 REGARDLESS of msg_adj.  This is verified host-side
by induction before the device kernel is chosen: starting from
log_msg_0 = 0 (trunc = 0 -> inc = 0 for any adjacency), each step is a
per-edge recurrence with constant coefficients; we simulate it and check
the (-1, 0] range at every step with a wide margin.  If the check ever
fails, a generic full-graph fallback is used instead.

The device kernel then only needs, per iteration:
  new  = halfC + 0.5*log_msg            (halfC: constant per edge/state)
  dd   = new0 - new1; lse = max + log1p(exp(-|dd|))
  log_msg' = new - lse
  segdiff = S_band^T @ dd               (banded one-hot matmul, edges
                                         pre-sorted by edge_out, output
                                         lands nodes-on-partitions)
All 15 iterations run from SBUF; prob_step, probs and the loss are
produced on-device.  The whole kernel is replicated over the 8 cores
(the 15-step recurrence is inherently serial and tiny, so no cross-core
split beats the ~5us/iter collective floor); core 0's output is
returned.
"""

import sys

sys.path.insert(0, "/opt/trn_rl_repo")

import numpy as np

N = 2048
E = 8192
ITERS = 15
DAMP = 0.5
P = 128
KT = E // P  # 64 K-tiles of 128 edges


# ----------------------------------------------------------------------------
# host-side reference pieces
# ----------------------------------------------------------------------------

def _lse2_cols(a, b):
    m = np.maximum(a, b)
    return m + np.log1p(np.exp(-np.abs(a - b)))


def _fast_sim(J, b, msg_node):
    """inc==0 recurrence, fp32, valid for ANY msg_adj while the range
    guard holds."""
    edge_in = msg_node[:, 0].astype(np.int64)
    edge_out = msg_node[:, 1].astype(np.int64)
    Je = J[edge_in, edge_out].astype(np.float32)
    b1 = b[:, 0].astype(np.float32)
    bin_ = b1[edge_in]
    w = Je + bin_
    z = bin_ - Je
    C0 = _lse2_cols(w, -w).astype(np.float32)
    C1 = _lse2_cols(z, -z).astype(np.float32)
    X = np.zeros((E, 2), np.float32)
    ok = True
    for _ in range(ITERS):
        new = 0.5 * np.stack([C0, C1], axis=1) + 0.5 * X
        lse = _lse2_cols(new[:, 0], new[:, 1])
        X = (new - lse[:, None]).astype(np.float32)
        if X.min() <= -0.98:
            ok = False
            break
    return ok


def _fallback(J, b, msg_node, msg_adj, mask, target):
    """Exact numpy port of the reference (host compute)."""
    J = J.astype(np.float32)
    edge_in = msg_node[:, 0].astype(np.int64)
    edge_out = msg_node[:, 1].astype(np.int64)
    log_phi = np.concatenate([b, -b], axis=1).astype(np.float32)
    state = np.array([[1.0, -1.0], [-1.0, 1.0]], np.float32)
    Je = J[edge_in, edge_out]
    log_psi = Je[:, None, None] * state[None]
    phi_in = log_phi[edge_in][:, None, :]
    adjT = msg_adj.T.astype(np.float32)

    def lse(x, ax):
        m = x.max(axis=ax, keepdims=True)
        return m + np.log(np.exp(x - m).sum(axis=ax, keepdims=True))

    log_msg = np.zeros((E, 2), np.float32)
    prob0 = []
    for _ in range(ITERS):
        inc = adjT @ np.trunc(log_msg)
        log_temp = log_psi + phi_in + inc[:, None, :]
        new = (1.0 - DAMP) * lse(mask * log_temp, 2)[:, :, 0] + DAMP * log_msg
        new = new - lse(new, 1)
        log_msg = new
        seg = np.zeros((N, 2), np.float32)
        np.add.at(seg, edge_out, new)
        lp = log_phi + seg
        lp = lp - lse(lp, 1)
        prob0.append(np.exp(lp[:, 0]))
    seg = np.zeros((N, 2), np.float32)
    np.add.at(seg, edge_out, log_msg)
    lp = log_phi + seg
    lp = lp - lse(lp, 1)
    prob_step = np.stack(prob0, axis=1)
    loss = np.sum(np.where(target > 0, target * np.log(np.maximum(target, 1e-38)), 0.0)
                  - target * lp).astype(np.float32)
    return np.exp(lp).astype(np.float32), np.float32(loss), prob_step.astype(np.float32)


# ----------------------------------------------------------------------------
# device kernel
# ----------------------------------------------------------------------------

def _build_device(nc, tc, wspecs, W):
    import concourse.bass as bass
    from concourse import mybir

    f32 = mybir.dt.float32
    AF = mybir.ActivationFunctionType
    OP = mybir.AluOpType
    nv, ns, nt_ = nc.vector, nc.scalar, nc.tensor

    je_d = nc.dram_tensor("je_pk", [P, KT], f32, kind="ExternalInput").ap()
    be_d = nc.dram_tensor("be_pk", [P, KT], f32, kind="ExternalInput").ap()
    sband_d = nc.dram_tensor("sband", [P, KT * W], f32, kind="ExternalInput").ap()
    bnode_d = nc.dram_tensor("bnode", [P, 16], f32, kind="ExternalInput").ap()
    targ_d = nc.dram_tensor("target_r", [P, 32], f32, kind="ExternalInput").ap()

    probs_o = nc.dram_tensor("probs_o", [P, 32], f32, kind="ExternalOutput").ap()
    loss_o = nc.dram_tensor("loss_o", [1, 1], f32, kind="ExternalOutput").ap()
    pstep_o = nc.dram_tensor("pstep_o", [P, 16 * ITERS], f32, kind="ExternalOutput").ap()

    with tc.tile_pool(name="cp", bufs=1) as cp, \
         tc.tile_pool(name="wp", bufs=2) as wp, \
         tc.tile_pool(name="pp", bufs=2, space="PSUM") as pp:

        # ---- constant loads -------------------------------------------------
        sband = cp.tile([P, KT * W], f32)
        nc.sync.dma_start(sband[:], sband_d[:])
        bn = cp.tile([P, 16], f32)
        nc.sync.dma_start(bn[:], bnode_d[:])
        targ = cp.tile([P, 32], f32)
        nc.sync.dma_start(targ[:], targ_d[:])
        je = cp.tile([P, KT], f32)
        nc.sync.dma_start(je[:], je_d[:])
        be = cp.tile([P, KT], f32)
        nc.sync.dma_start(be[:], be_d[:])

        # ---- constants: halfC[p, kt, s] = 0.5 * LSE2(+-Je + b_in) ----------
        halfC = cp.tile([P, KT, 2], f32)
        wv = cp.tile([P, KT], f32)
        zv = cp.tile([P, KT], f32)
        nv.tensor_tensor(out=wv[:], in0=je[:], in1=be[:], op=OP.add)
        nv.tensor_tensor(out=zv[:], in0=be[:], in1=je[:], op=OP.subtract)
        for src, slot in ((wv, 0), (zv, 1)):
            aw = cp.tile([P, KT], f32, tag="c_aw")
            ns.activation(aw[:], src[:], AF.Abs)
            ew = cp.tile([P, KT], f32, tag="c_ew")
            ns.activation(ew[:], aw[:], AF.Exp, scale=-2.0)
            nv.tensor_scalar_add(ew[:], ew[:], 1.0)
            lw = cp.tile([P, KT], f32, tag="c_lw")
            ns.activation(lw[:], ew[:], AF.Ln)
            nv.tensor_tensor(out=lw[:], in0=lw[:], in1=aw[:], op=OP.add)
            nv.tensor_scalar_mul(halfC[:, :, slot], lw[:], 0.5)

        z2 = cp.tile([P, 16], f32)
        nv.tensor_scalar_mul(z2[:], bn[:], 2.0)

        zw = cp.tile([P, P], f32)
        nv.memset(zw[:], 0.0)

        sd_all = cp.tile([P, ITERS * 16], f32)

        X = wp.tile([P, KT, 2], f32, tag="X")
        nv.memset(X[:], 0.0)

        # ---- 15 iterations --------------------------------------------------
        for t in range(ITERS):
            q = wp.tile([P, KT, 2], f32, tag="q")
            nv.tensor_scalar_mul(q[:], X[:], 0.5)
            new = wp.tile([P, KT, 2], f32, tag="new")
            nv.tensor_tensor(out=new[:], in0=q[:], in1=halfC[:], op=OP.add)
            dd = wp.tile([P, KT], f32, tag="dd")
            nv.tensor_tensor(out=dd[:], in0=new[:, :, 0], in1=new[:, :, 1],
                             op=OP.subtract)
            mx = wp.tile([P, KT], f32, tag="mx")
            nv.tensor_tensor(out=mx[:], in0=new[:, :, 0], in1=new[:, :, 1],
                             op=OP.max)
            ab = wp.tile([P, KT], f32, tag="ab")
            ns.activation(ab[:], dd[:], AF.Abs)
            ex = wp.tile([P, KT], f32, tag="ex")
            ns.activation(ex[:], ab[:], AF.Exp, scale=-1.0)
            nv.tensor_scalar_add(ex[:], ex[:], 1.0)
            ll = wp.tile([P, KT], f32, tag="ll")
            ns.activation(ll[:], ex[:], AF.Ln)
            nv.tensor_tensor(out=ll[:], in0=ll[:], in1=mx[:], op=OP.add)
            Xn = wp.tile([P, KT, 2], f32, tag="X")
            lse_b = ll[:].rearrange("p (a o) -> p a o", o=1).to_broadcast([P, KT, 2])
            nv.tensor_tensor(out=Xn[:], in0=new[:], in1=lse_b, op=OP.subtract)

            # segdiff: S_band^T @ dd -> [128 nodes, 16 blocks] psum
            seg_ps = pp.tile([P, 16], f32, tag="seg")
            nt_.matmul(seg_ps[:, 0:16], lhsT=zw[:], rhs=dd[:, 0:16],
                       start=True, stop=False, skip_group_check=True)
            nmm = len(wspecs)
            for i, (kt, boff, bp, blk, wd) in enumerate(wspecs):
                nt_.matmul(seg_ps[bp:bp + wd, blk:blk + 1],
                           lhsT=sband[:, kt * W + boff:kt * W + boff + wd],
                           rhs=dd[:, kt:kt + 1],
                           start=False, stop=(i == nmm - 1),
                           skip_group_check=True,
                           tile_position=(0, bp))
            ns.copy(sd_all[:, t * 16:(t + 1) * 16], seg_ps[:])
            X = Xn

        # ---- prob_step ------------------------------------------------------
        # sd_all[p, t*16+nt] = segdiff_t[node nt*128+p]
        zzt = cp.tile([P, ITERS * 16], f32)
        z2b = z2[:].rearrange("p (o nt) -> p o nt", o=1).to_broadcast([P, ITERS, 16])
        nv.tensor_tensor(out=zzt[:].rearrange("p (t nt) -> p t nt", t=ITERS),
                         in0=sd_all[:].rearrange("p (t nt) -> p t nt", t=ITERS),
                         in1=z2b, op=OP.add)
        emt = cp.tile([P, ITERS * 16], f32)
        ns.activation(emt[:], zzt[:], AF.Exp, scale=-1.0)
        nv.tensor_scalar_add(emt[:], emt[:], 1.0)
        pst = cp.tile([P, ITERS * 16], f32)
        nv.reciprocal(pst[:], emt[:])
        nc.sync.dma_start(pstep_o[:], pst[:])

        # ---- final probs + loss --------------------------------------------
        # zd = zz at t = ITERS-1;  probs = [sigmoid(zd), sigmoid(-zd)]
        zd = zzt[:, (ITERS - 1) * 16:ITERS * 16]
        em = cp.tile([P, 16], f32, tag="f_em")
        ns.activation(em[:], zd, AF.Exp, scale=-1.0)
        ep = cp.tile([P, 16], f32, tag="f_ep")
        ns.activation(ep[:], zd, AF.Exp)
        a0 = cp.tile([P, 16], f32, tag="f_a0")
        nv.tensor_scalar_add(a0[:], em[:], 1.0)
        a1 = cp.tile([P, 16], f32, tag="f_a1")
        nv.tensor_scalar_add(a1[:], ep[:], 1.0)
        pr = cp.tile([P, 16, 2], f32, tag="f_pr")
        nv.reciprocal(pr[:, :, 0], a0[:])
        nv.reciprocal(pr[:, :, 1], a1[:])
        nc.sync.dma_start(probs_o[:], pr[:].rearrange("p a b -> p (a b)"))

        # loss = sum target * (ln(target) + L_s),  L_0 = ln(1+em), L_1 = ln(1+ep)
        L = cp.tile([P, 16, 2], f32, tag="f_L")
        ns.activation(L[:, :, 0], a0[:], AF.Ln)
        ns.activation(L[:, :, 1], a1[:], AF.Ln)
        lnt = cp.tile([P, 32], f32, tag="f_lnt")
        ns.activation(lnt[:], targ[:], AF.Ln)
        s1 = cp.tile([P, 32], f32, tag="f_s1")
        nv.tensor_tensor(out=s1[:], in0=lnt[:],
                         in1=L[:].rearrange("p a b -> p (a b)"), op=OP.add)
        nv.tensor_tensor(out=s1[:], in0=s1[:], in1=targ[:], op=OP.mult)
        rs = cp.tile([P, 1], f32, tag="f_rs")
        nv.tensor_reduce(out=rs[:], in_=s1[:], axis=mybir.AxisListType.X,
                         op=OP.add)
        ones = cp.tile([P, 1], f32, tag="f_ones")
        nv.memset(ones[:], 1.0)
        ls_ps = pp.tile([1, 8], f32, tag="loss")
        nt_.matmul(ls_ps[0:1, 0:1], lhsT=ones[:], rhs=rs[:], start=True,
                   stop=True, skip_group_check=True)
        ls_sb = cp.tile([1, 1], f32, tag="f_ls")
        ns.copy(ls_sb[:], ls_ps[0:1, 0:1])
        nc.sync.dma_start(loss_o[:], ls_sb[:])


def _run_device(host_inputs, trace=False):
    import concourse.bacc as bacc
    import concourse.tile as tile
    from concourse import bass_utils

    wspecs = host_inputs.pop("_wspecs")
    W = host_inputs.pop("_W")

    nc = bacc.Bacc("TRN2", target_bir_lowering=False, debug=False)
    with tile.TileContext(nc) as tc:
        _build_device(nc, tc, wspecs, W)
    nc.compile()

    n_cores = 8
    in_maps = [dict(host_inputs) for _ in range(n_cores)]
    try:
        res = bass_utils.run_bass_kernel_spmd(
            nc, in_maps, core_ids=list(range(n_cores)), trace=trace)
    except ModuleNotFoundError:
        # NTFF profile hook unavailable in this environment
        res = bass_utils.run_bass_kernel_spmd(
            nc, in_maps, core_ids=list(range(n_cores)), trace=False)
    res.nc = nc
    res.in_maps = in_maps
    return res


def _prep_host(J, b, msg_node):
    edge_in = msg_node[:, 0].astype(np.int64)
    edge_out = msg_node[:, 1].astype(np.int64)
    perm = np.argsort(edge_out, kind="stable")
    ein_s = edge_in[perm]
    eo_s = edge_out[perm]

    # host-side gathers: pure index-based data movement (no arithmetic)
    je_pk = np.ascontiguousarray(
        J.astype(np.float32)[ein_s, eo_s].reshape(KT, P).T)
    be_pk = np.ascontiguousarray(
        b[:, 0].astype(np.float32)[ein_s].reshape(KT, P).T)

    los = eo_s.reshape(KT, P)[:, 0]
    his = eo_s.reshape(KT, P)[:, -1]

    def place(lo, hi):
        """Window [lo, hi] within one 128-node block -> (base, width) with
        matmul tile_position validity: width<=32 -> base 32-aligned,
        width<=64 -> base 64-aligned, else base = block start."""
        for strip in (32, 64, 128):
            base = (lo // strip) * strip
            if hi - base + 1 <= strip:
                return base, hi - base + 1
        raise AssertionError

    # window specs: (kt, band_col_off, psum_base_partition, node_block, width)
    # split raw windows at 128-node block boundaries first
    raw = []   # per kt: list of (base, width, blk)
    cbs = []   # sband column base per kt
    for kt in range(KT):
        lo, hi = int(los[kt]), int(his[kt])
        parts = []
        while lo // P != hi // P:
            mid = (lo // P + 1) * P - 1
            parts.append((lo, mid))
            lo = mid + 1
        parts.append((lo, hi))
        placed = [place(a, b) + (a // P,) for a, b in parts]  # (abs_base, wd, blk)
        raw.append(placed)
        cbs.append(placed[0][0])

    W = int(max(his[kt] - cbs[kt] + 1 for kt in range(KT)))
    if W > 256:
        return None
    sband = np.zeros((P, KT * W), np.float32)
    col = (eo_s.reshape(KT, P) - np.asarray(cbs)[:, None])  # [KT, P]
    for kt in range(KT):
        sband[np.arange(P), kt * W + col[kt]] = 1.0

    wspecs = []
    for kt in range(KT):
        for base, wd, blk in raw[kt]:
            wspecs.append((kt, base - cbs[kt], base % P, blk, wd))

    b1 = b[:, 0].astype(np.float32)
    bnode = np.ascontiguousarray(b1.reshape(16, P).T)
    return {
        "je_pk": je_pk,
        "be_pk": be_pk,
        "sband": sband,
        "bnode": bnode,
        "_wspecs": wspecs,
        "_W": W,
    }


def _unpack_outputs(outs):
    probs = outs["probs_o"].reshape(P, 16, 2).transpose(1, 0, 2).reshape(N, 2)
    loss = np.float32(outs["loss_o"].reshape(-1)[0])
    pstep = outs["pstep_o"].reshape(P, ITERS, 16).transpose(2, 0, 1).reshape(N, ITERS)
    return (np.ascontiguousarray(probs), loss, np.ascontiguousarray(pstep))


def kernel(J, b, msg_node, msg_adj, mask, target, _trace=False, _return_res=False):
    J = np.asarray(J)
    b = np.asarray(b)
    msg_node = np.asarray(msg_node)
    mask = np.asarray(mask)
    target = np.asarray(target)

    ok = bool(np.all(np.asarray(mask) == 1.0))
    if ok:
        ok = _fast_sim(J, b, msg_node)
    host_inputs = _prep_host(J, b, msg_node) if ok else None
    if host_inputs is None:
        return _fallback(J, b, msg_node, np.asarray(msg_adj), mask, target)

    host_inputs["target_r"] = np.ascontiguousarray(
        target.astype(np.float32).reshape(16, P, 2).transpose(1, 0, 2).reshape(P, 32))

    res = _run_device(host_inputs, trace=_trace)
    out = _unpack_outputs(res.results[0])
    if _return_res:
        return out, res
    return out


if __name__ == "__main__":
    import reference

    inputs = {k: np.asarray(v) for k, v in reference.setup_inputs().items()}
    out = kernel(**inputs)
    print([np.asarray(o).shape for o in out])
